# revision 12
# baseline (speedup 1.0000x reference)
"""Trainium2 Bass kernel for Keras-style CTC batch loss.

Problem: y_pred [256, 256, 512] f32 softmax probs, y_true [256, 64] int64
labels (0=pad, blank=511). Output [256, 1] f32 negative log-likelihood.

Strategy (pure data parallel, 32 samples per core on 8 cores):
  - Host precomputes per-sample extended-label gather indices and skip
    masks from y_true (tiny); y_pred streams through the device untouched.
  - Device: stream y_pred (16.8 MB/core) as 32 x 512KB loads (2 t-blocks
    per load, 4D strided AP so partitions stay (sample, t%16)); GPSIMD
    ap_gather pulls 2x144 extended-state probabilities per load; ACT casts
    to bf16 with the reference's +1e-7 fused; DMA collapses gather tiles
    into 4 phase-group stores [64 rows, 32 slots x 144].
  - DVE runs the serial CTC recursion in linear space (no per-step
    transcendentals): 127 slots x 4 bf16 tensor_tensor ops on [64, 132].
    Rows 0..31: forward alpha from t=0..127. Rows 32..63: backward
    adjoint g from t=255..128 with the state axis flipped about s=128 so
    both directions share identical shifted-view ops. Periodic max-renorm
    (every 6 slots) keeps fp range; log-normalizers accumulate on [64, 1].
    Phase-group tiles let the recursion overlap the stream.
  - Combine: one extra band apply on the forward rows, reversed dot with
    the backward rows, ln on ACT, negate, DMA out.

Self-contained: shapes/sharding hardcoded; no problem files read.
"""

import numpy as np
import ml_dtypes
from contextlib import ExitStack

import concourse.bass as bass
import concourse.tile as tile
from concourse import bacc, mybir
from concourse.bass_utils import run_bass_kernel_spmd

# ---------------- problem constants ----------------
B, T, C, L = 256, 256, 512, 64
S = 2 * L + 1          # 129 extended states
BLANK = C - 1          # 511
EPS = 1e-7
NCORES = 8
SPC = B // NCORES      # 32 samples per core
R = 2 * SPC            # 64 recursion rows (fwd + bwd)
W = 144                # slot width (mult of 16, >= S)
FD = 132               # recursion free size (states j = 0..131)
NSLOT = 127            # recursion slots
JB = 2                 # t-blocks (16 t each) per load/gather
NELEM = 516            # stage width per t-block incl zero columns
ZCOL = 512             # stage column with exact 0.0 (invalid states)
NGRP = 4               # phase groups; each covers JB*16 = 32 slots
NLOAD = NGRP * 8       # 32 loads of [8 samp, 32 t, 512c] = 512KB
NIDX = JB * W          # 288 gather indices per call
NSTAGE = 12            # stage buffers (no tight WAR reuse)
RENORM_SLOTS = tuple(range(3, NSLOT - 2, 6))  # scale applied at slot+2
LN2 = float(np.log(2.0))
F32 = mybir.dt.float32
BF16 = mybir.dt.bfloat16
I16 = mybir.dt.int16
I32 = mybir.dt.int32
U32 = mybir.dt.uint32
AF = mybir.ActivationFunctionType
ALU = mybir.AluOpType


# ---------------- host-side tables ----------------
def _ext_and_mask(labels):
    l = int(np.count_nonzero(labels))
    ext = np.full(S, BLANK, np.int64)
    ext[1::2] = np.asarray(labels, np.int64)
    m = np.zeros(S, np.float32)
    s = np.arange(2, S)
    mm = (ext[2:] != BLANK) & (ext[2:] != ext[:-2])
    m[s] = mm.astype(np.float32)
    return ext, m, l


def _wrap_idx(idx_row):
    """[NIDX] -> ap_gather wrapped layout [16, NIDX//16] for one core."""
    return np.asarray(idx_row, np.int16).reshape(NIDX // 16, 16).T


def _core_tables(y_true_core):
    """Per-core device tables from labels [SPC, L].

    idx_sb  [128, 8*(NIDX//16)] int16 : per-octet wrapped gather indices
    msrc    [R, W]   bf16             : source-side skip mask per row
    ind     [R, W]   bf16             : init indicator per row
    """
    idx_rows = np.full((R, W), ZCOL, np.int16)
    msrc = np.zeros((R, W), np.float32)
    ind = np.zeros((R, W), np.float32)
    for b in range(SPC):
        ext, m, l = _ext_and_mask(y_true_core[b])
        idx_rows[b, :2 * l + 1] = ext[:2 * l + 1]
        msrc[b, : S - 2] = m[2:]
        ind[b, 0] = 1.0
        ind[b, 1] = 1.0
        r = SPC + b
        for i in range(S):
            s = 128 - i
            if s <= 2 * l:
                idx_rows[r, i] = ext[s]
        sidx = 128 - np.arange(W)
        ok = (sidx >= 2) & (sidx <= S - 1)
        msrc[r, ok] = m[sidx[ok]]
        ind[r, 128 - 2 * l] = 1.0
        ind[r, 129 - 2 * l] = 1.0
    # 2-block index rows: block j gathers from stage cols [j*516, j*516+516)
    ncol = NIDX // 16
    idx_sb = np.zeros((128, 8 * ncol), np.int16)
    for o in range(8):
        blocks = []
        for g in range(8):
            row2 = np.concatenate(
                [idx_rows[8 * o + g] + j * NELEM for j in range(JB)])
            blocks.append(_wrap_idx(row2))
        idx_sb[:, ncol * o: ncol * (o + 1)] = np.concatenate(blocks, axis=0)
    return (idx_sb,
            msrc.astype(ml_dtypes.bfloat16),
            ind.astype(ml_dtypes.bfloat16))


# ---------------- device kernel ----------------
def _emit(nc):
    yp = nc.dram_tensor("yp", [SPC, T, C], F32, kind="ExternalInput")
    idx_d = nc.dram_tensor("idx", [128, 8 * (NIDX // 16)], I16,
                           kind="ExternalInput")
    msrc_d = nc.dram_tensor("msrc", [R, W], BF16, kind="ExternalInput")
    ind_d = nc.dram_tensor("ind", [R, W], BF16, kind="ExternalInput")
    out_d = nc.dram_tensor("loss_out", [SPC, 1], F32, kind="ExternalOutput")

    with tile.TileContext(nc) as tc, ExitStack() as ctx:
        consts = ctx.enter_context(tc.tile_pool(name="consts", bufs=1))
        stage_p = ctx.enter_context(tc.tile_pool(name="stage", bufs=1))
        # one gout/gbf buffer per stream iteration: zero WAR reuse, so the
        # GPSIMD/ACT stream instructions never wait on downstream consumers
        gout_p = ctx.enter_context(tc.tile_pool(name="gout", bufs=NLOAD))
        gbf_p = ctx.enter_context(tc.tile_pool(name="gbf", bufs=NLOAD))
        pstore_p = ctx.enter_context(tc.tile_pool(name="pstore", bufs=NGRP))
        state_p = ctx.enter_context(tc.tile_pool(name="state", bufs=1))
        tmp_p = ctx.enter_context(tc.tile_pool(name="tmp", bufs=3))

        idx_sb = consts.tile([128, 8 * (NIDX // 16)], I16)
        msrc = consts.tile([R, W], BF16)
        ind = consts.tile([R, W], BF16)
        nc.sync.dma_start(idx_sb[:, :], idx_d.ap())
        nc.sync.dma_start(msrc[:, :], msrc_d.ap())
        nc.sync.dma_start(ind[:, :], ind_d.ap())

        # phase-group stores: group g holds slots [32g, 32g+32) x W
        pgroup = [pstore_p.tile([R, JB * 16 * W], BF16, tag="pstore",
                                name=f"pg{g}") for g in range(NGRP)]

        # persistent stage buffers; zero pad columns memset once
        stages = [stage_p.tile([128, JB * NELEM], F32, name=f"stg{i}")
                  for i in range(NSTAGE)]
        for st in stages:
            zv = st[:, :].rearrange("p (j c) -> p j c", j=JB, c=NELEM)
            nc.gpsimd.memset(zv[:, :, ZCOL:NELEM], 0.0)

        # ---- stream: load -> gather -> cast -> collapse ----
        for g in range(NGRP):
            for o in range(8):
                it = g * 8 + o
                st = stages[it % NSTAGE]
                for j in range(JB):
                    if o < 4:
                        # fwd rows: samples 8o..8o+8, t ascending from 32g
                        t0 = JB * 16 * g + 16 * j
                        src = yp.ap()[8 * o: 8 * o + 8, t0: t0 + 16, :]
                    else:
                        # bwd rows: samples 8(o-4).., t desc. from 255-32g-16j
                        b0 = 8 * (o - 4)
                        hi = T - 1 - JB * 16 * g - 16 * j
                        src = yp.ap()[b0: b0 + 8, hi: hi - 16: -1, :]
                    nc.sync.dma_start(
                        st[:, j * NELEM: j * NELEM + C], src)
                go = gout_p.tile([128, NIDX], F32, tag="gout")
                nc.gpsimd.ap_gather(
                    out_ap=go[:, :], in_ap=st[:, :],
                    idxs_ap=idx_sb[:, (NIDX // 16) * o: (NIDX // 16) * (o + 1)],
                    channels=128, num_elems=JB * NELEM, d=1, num_idxs=NIDX)
                gb = gbf_p.tile([128, NIDX], BF16, tag="gbf")
                nc.scalar.activation(gb[:, :], go[:, :], AF.Copy, bias=EPS)
                # collapse [128, (j c)] -> rows ro..ro+8; within-group slot
                # position w' = 2*ti + j keeps (j c) contiguous on the dst
                # so the DMA stays 3-dim.
                # SWDGE (gpsimd) keeps collapse completions on the DMASW
                # lanes, so the HWDGE lane-guards never couple future loads
                # to slow SBUF->SBUF collapse completions.
                ro = 8 * (o % 4) + (0 if o < 4 else SPC)
                dst = pgroup[g][ro: ro + 8, :].rearrange(
                    "s (ti jc) -> s ti jc", ti=16, jc=JB * W)
                nc.gpsimd.dma_start(dst, gb[:, :])

        # ---- recursion state ----
        bufA = state_p.tile([R, 2 + W], BF16)
        bufB = state_p.tile([R, 2 + W], BF16)
        racc = state_p.tile([R, 1], F32)
        mx = state_p.tile([R, 1], F32)
        rinv = state_p.tile([R, 1], F32)
        ebi = state_p.tile([R, 1], U32)
        ebf = state_p.tile([R, 1], F32)
        bconst = state_p.tile([R, 1], F32)
        nc.vector.memset(bconst[:, :], float(163 * LN2))

        nc.vector.memset(bufA[:, :], 0.0)
        nc.vector.memset(bufB[:, :], 0.0)
        nc.vector.memset(racc[:, :], 0.0)

        # init: x = slot-0 p-vector * ind
        nc.vector.tensor_tensor(bufA[:, 2:2 + W], pgroup[0][:, 0:W],
                                ind[:, :], ALU.mult)

        bufs = (bufA, bufB)
        scale_slots = {k + 2 for k in RENORM_SLOTS}
        for k in range(NSLOT):
            src_b = bufs[k % 2]
            dst_b = bufs[1 - k % 2]
            blk = k + 1
            gph, w = divmod(blk, JB * 16)
            bi = 2 * (w % 16) + (w // 16)   # w' = 2*ti + j slot order
            psl = pgroup[gph][:, bi * W: bi * W + FD]
            t_t = tmp_p.tile([R, FD], BF16, tag="t")
            u_t = tmp_p.tile([R, FD], BF16, tag="u")
            w_t = tmp_p.tile([R, FD], BF16, tag="w")
            nc.vector.tensor_tensor(t_t[:, :], src_b[:, 0:FD],
                                    msrc[:, 0:FD], ALU.mult)
            nc.vector.tensor_tensor(u_t[:, :], src_b[:, 2:2 + FD],
                                    src_b[:, 1:1 + FD], ALU.add)
            nc.vector.tensor_tensor(w_t[:, :], u_t[:, :], t_t[:, :], ALU.add)
            if k in scale_slots:
                # fold the pending renorm scale into the p-multiply
                nc.vector.scalar_tensor_tensor(dst_b[:, 2:2 + FD], w_t[:, :],
                                               rinv[:, :], psl,
                                               ALU.mult, ALU.mult)
            else:
                nc.vector.tensor_tensor(dst_b[:, 2:2 + FD], w_t[:, :], psl,
                                        ALU.mult)
            if k in RENORM_SLOTS:
                # pow2 bookkeeping: eb = raw exponent field of mx; racc
                # accumulates raw eb (the 163 = 127+36 renorm-target bias is
                # removed once at the end); rinv = exp(-ln2*(eb-163)) via the
                # Exp bias, applied at slot k+2. Renorm target 2^36 keeps the
                # in-row dynamic range clear of bf16 flush-to-zero.
                nc.vector.tensor_reduce(mx[:, :], dst_b[:, 2:2 + FD],
                                        axis=mybir.AxisListType.X, op=ALU.max)
                nc.vector.tensor_scalar(ebi[:, :],
                                        mx[:, :].bitcast(U32),
                                        23, None, ALU.logical_shift_right)
                nc.vector.tensor_copy(ebf[:, :], ebi[:, :])
                nc.vector.tensor_tensor(racc[:, :], racc[:, :], ebf[:, :],
                                        ALU.add)
                nc.scalar.activation(rinv[:, :], ebf[:, :], AF.Exp,
                                     scale=-LN2, bias=bconst[:, :])

        fin = bufs[NSLOT % 2]          # holds a_127 (fwd) / g_128 (bwd)

        # ---- combine ----
        zt = tmp_p.tile([SPC, FD], BF16, tag="t")
        zu = tmp_p.tile([SPC, FD], BF16, tag="u")
        zz = tmp_p.tile([SPC, FD], BF16, tag="w")
        nc.vector.tensor_tensor(zt[:, :], fin[0:SPC, 0:FD],
                                msrc[0:SPC, 0:FD], ALU.mult)
        nc.vector.tensor_tensor(zu[:, :], fin[0:SPC, 2:2 + FD],
                                fin[0:SPC, 1:1 + FD], ALU.add)
        nc.vector.tensor_tensor(zz[:, :], zu[:, :], zt[:, :], ALU.add)

        grev = state_p.tile([SPC, S], BF16)
        raccB = state_p.tile([SPC, 1], F32)
        # reversed copy of bwd rows into fwd partitions: grev[b, s] = g[b, 128-s]
        nc.sync.dma_start(grev[:, :], fin[SPC:R, 2 + 128: 2 - 1: -1])
        nc.sync.dma_start(raccB[:, :], racc[SPC:R, :])

        # log-space combine with exponent/mantissa decomposition (the ACT
        # Ln LUT clamps below ~1e-20, so ln args must stay in [1, 2)):
        #   ln v = LN2 * (expbits(v) - 127) + Ln(mantissa(v))
        # zero entries get a -1e18 penalty so they drop out of logsumexp.
        def exact_ln(src_bf16, pname):
            ebu = state_p.tile([SPC, S], mybir.dt.uint16, name=f"{pname}_ebu")
            ebv = state_p.tile([SPC, S], F32, name=f"{pname}_eb")
            mnt = state_p.tile([SPC, S], mybir.dt.uint16, name=f"{pname}_mn")
            lnm = state_p.tile([SPC, S], F32, name=f"{pname}_lnm")
            pen = state_p.tile([SPC, S], F32, name=f"{pname}_pen")
            lnv = state_p.tile([SPC, S], F32, name=f"{pname}_ln")
            bits = src_bf16.bitcast(mybir.dt.uint16)
            nc.vector.tensor_scalar(ebu[:, :], bits, 7, None,
                                    ALU.logical_shift_right)
            nc.vector.tensor_copy(ebv[:, :], ebu[:, :])
            nc.vector.tensor_scalar(mnt[:, :], bits, 0x7F, None,
                                    ALU.bitwise_and)
            nc.vector.tensor_scalar(mnt[:, :], mnt[:, :], 0x3F80, None,
                                    ALU.bitwise_or)
            nc.scalar.activation(lnm[:, :], mnt[:, :].bitcast(BF16), AF.Ln)
            nc.vector.tensor_scalar(pen[:, :], bits, 0, -1e18,
                                    ALU.is_equal, ALU.mult)
            nc.vector.tensor_scalar(ebv[:, :], ebv[:, :], LN2, -127.0 * LN2,
                                    ALU.mult, ALU.add)
            nc.vector.tensor_tensor(lnv[:, :], ebv[:, :], lnm[:, :], ALU.add)
            nc.vector.tensor_tensor(lnv[:, :], lnv[:, :], pen[:, :], ALU.add)
            return lnv

        lnz = exact_ln(zz[:, 0:S], "z")
        lng = exact_ln(grev[:, :], "g")
        sums = state_p.tile([SPC, S], F32)
        m_t = state_p.tile([SPC, 1], F32)
        negm = state_p.tile([SPC, 1], F32)
        e_t = state_p.tile([SPC, S], F32)
        dot = state_p.tile([SPC, 1], F32)
        nc.vector.tensor_tensor(sums[:, :], lnz[:, :], lng[:, :], ALU.add)
        nc.vector.tensor_scalar_max(sums[:, :], sums[:, :], -1e18)
        nc.vector.tensor_reduce(m_t[:, :], sums[:, :],
                                axis=mybir.AxisListType.X, op=ALU.max)
        nc.vector.tensor_scalar_mul(negm[:, :], m_t[:, :], -1.0)
        nc.scalar.activation(e_t[:, :], sums[:, :], AF.Exp, bias=negm[:, :])
        nc.vector.tensor_reduce(dot[:, :], e_t[:, :],
                                axis=mybir.AxisListType.X, op=ALU.add)
        lnd = state_p.tile([SPC, 1], F32)
        nc.scalar.activation(lnd[:, :], dot[:, :], AF.Ln)
        # loss = -(ln(dot) + m + LN2*(raccF + raccB))
        s1 = state_p.tile([SPC, 1], F32)
        s2 = state_p.tile([SPC, 1], F32)
        s3 = state_p.tile([SPC, 1], F32)
        loss = state_p.tile([SPC, 1], F32)
        nc.vector.tensor_tensor(s1[:, :], racc[0:SPC, :], raccB[:, :], ALU.add)
        # remove the 163 renorm-target bias for all 2*NREN accumulations
        nc.vector.tensor_scalar(s1[:, :], s1[:, :], LN2,
                                -2.0 * len(RENORM_SLOTS) * 163 * LN2,
                                ALU.mult, ALU.add)
        nc.vector.tensor_tensor(s2[:, :], lnd[:, :], m_t[:, :], ALU.add)
        nc.vector.tensor_tensor(s3[:, :], s2[:, :], s1[:, :], ALU.add)
        nc.vector.tensor_scalar_mul(loss[:, :], s3[:, :], -1.0)
        nc.sync.dma_start(out_d.ap(), loss[:, :])
    return nc


_NC_CACHE = None


def _build():
    global _NC_CACHE
    if _NC_CACHE is None:
        nc = bacc.Bacc("TRN2", target_bir_lowering=False, debug=False,
                       enable_asserts=False)
        _emit(nc)
        nc.compile()
        _NC_CACHE = nc
    return _NC_CACHE


def kernel(y_true, y_pred):
    y_true = np.asarray(y_true)
    y_pred = np.ascontiguousarray(np.asarray(y_pred, np.float32))
    nc = _build()
    in_maps = []
    for c in range(NCORES):
        sl = slice(c * SPC, (c + 1) * SPC)
        idx_sb, msrc, ind = _core_tables(y_true[sl])
        in_maps.append(dict(yp=y_pred[sl], idx=idx_sb, msrc=msrc, ind=ind))
    res = run_bass_kernel_spmd(nc, in_maps, core_ids=list(range(NCORES)))
    loss = np.concatenate([res.results[c]["loss_out"] for c in range(NCORES)],
                          axis=0)
    return loss.astype(np.float32)


# revision 14
# speedup vs baseline: 1.4694x; 1.4694x over previous
"""Trainium2 Bass kernel for Keras-style CTC batch loss.

Problem: y_pred [256, 256, 512] f32 softmax probs, y_true [256, 64] int64
labels (0=pad, blank=511). Output [256, 1] f32 negative log-likelihood.

Strategy (pure data parallel, 32 samples per core on 8 cores):
  - Host precomputes per-sample extended-label gather indices and skip
    masks from y_true (tiny); y_pred streams through the device untouched.
  - Device: stream y_pred (16.8 MB/core) as 32 x 512KB loads (2 t-blocks
    per load, 4D strided AP so partitions stay (sample, t%16)); GPSIMD
    ap_gather pulls 2x144 extended-state probabilities per load; ACT casts
    to bf16 with the reference's +1e-7 fused; DMA collapses gather tiles
    into 4 phase-group stores [64 rows, 32 slots x 144].
  - DVE runs the serial CTC recursion in linear space (no per-step
    transcendentals): 127 slots x 4 bf16 tensor_tensor ops on [64, 132].
    Rows 0..31: forward alpha from t=0..127. Rows 32..63: backward
    adjoint g from t=255..128 with the state axis flipped about s=128 so
    both directions share identical shifted-view ops. Periodic max-renorm
    (every 6 slots) keeps fp range; log-normalizers accumulate on [64, 1].
    Phase-group tiles let the recursion overlap the stream.
  - Combine: one extra band apply on the forward rows, reversed dot with
    the backward rows, ln on ACT, negate, DMA out.

Self-contained: shapes/sharding hardcoded; no problem files read.
"""

import numpy as np
import ml_dtypes
from contextlib import ExitStack

import concourse.bass as bass
import concourse.tile as tile
from concourse import bacc, mybir
from concourse.bass_utils import run_bass_kernel_spmd

# ---------------- problem constants ----------------
B, T, C, L = 256, 256, 512, 64
S = 2 * L + 1          # 129 extended states
BLANK = C - 1          # 511
EPS = 1e-7
NCORES = 8
SPC = B // NCORES      # 32 samples per core
R = 2 * SPC            # 64 recursion rows (fwd + bwd)
W = 144                # slot width (mult of 16, >= S)
FD = 132               # recursion free size (states j = 0..131)
NSLOT = 127            # recursion slots
JB = 2                 # t-blocks (16 t each) per load/gather
NELEM = 516            # stage width per t-block incl zero columns
ZCOL = 512             # stage column with exact 0.0 (invalid states)
NGRP = 4               # phase groups; each covers JB*16 = 32 slots
NLOAD = NGRP * 8       # 32 loads of [8 samp, 32 t, 512c] = 512KB
WL = 72                # odd (label) states gathered per t-block
NIDX = JB * WL         # 144 gather indices per call (labels only)
NSTAGE = 12            # stage buffers (no tight WAR reuse)
RENORM_SLOTS = tuple(range(3, NSLOT - 2, 6))  # scale applied at slot+2
LN2 = float(np.log(2.0))
F32 = mybir.dt.float32
BF16 = mybir.dt.bfloat16
I16 = mybir.dt.int16
I32 = mybir.dt.int32
U32 = mybir.dt.uint32
AF = mybir.ActivationFunctionType
ALU = mybir.AluOpType


# ---------------- host-side tables ----------------
def _ext_and_mask(labels):
    l = int(np.count_nonzero(labels))
    ext = np.full(S, BLANK, np.int64)
    ext[1::2] = np.asarray(labels, np.int64)
    m = np.zeros(S, np.float32)
    s = np.arange(2, S)
    mm = (ext[2:] != BLANK) & (ext[2:] != ext[:-2])
    m[s] = mm.astype(np.float32)
    return ext, m, l


def _wrap_idx(idx_row):
    """[NIDX] -> ap_gather wrapped layout [16, NIDX//16] for one core."""
    return np.asarray(idx_row, np.int16).reshape(NIDX // 16, 16).T


def _core_tables(y_true_core):
    """Per-core device tables from labels [SPC, L].

    idx_sb  [128, 8*(NIDX//16)] int16 : per-octet wrapped gather indices
    msrc    [R, W]   bf16             : source-side skip mask per row
    ind     [R, W]   bf16             : init indicator per row
    """
    idx_rows = np.full((R, W), ZCOL, np.int16)
    msrc = np.zeros((R, W), np.float32)
    ind = np.zeros((R, W), np.float32)
    for b in range(SPC):
        ext, m, l = _ext_and_mask(y_true_core[b])
        idx_rows[b, :2 * l + 1] = ext[:2 * l + 1]
        msrc[b, : S - 2] = m[2:]
        ind[b, 0] = 1.0
        ind[b, 1] = 1.0
        r = SPC + b
        for i in range(S):
            s = 128 - i
            if s <= 2 * l:
                idx_rows[r, i] = ext[s]
        sidx = 128 - np.arange(W)
        ok = (sidx >= 2) & (sidx <= S - 1)
        msrc[r, ok] = m[sidx[ok]]
        ind[r, 128 - 2 * l] = 1.0
        ind[r, 129 - 2 * l] = 1.0
    # 2-block odd-state (label) index rows: block j gathers labels from
    # stage cols [j*516, j*516+516); blank/even states are extracted by a
    # plain DMA of column 511 and broadcast on ACT.
    ncol = NIDX // 16
    idx_sb = np.zeros((128, 8 * ncol), np.int16)
    for o in range(8):
        blocks = []
        for g in range(8):
            odd = idx_rows[8 * o + g][1::2][:WL]
            row2 = np.concatenate([odd + j * NELEM for j in range(JB)])
            blocks.append(_wrap_idx(row2))
        idx_sb[:, ncol * o: ncol * (o + 1)] = np.concatenate(blocks, axis=0)
    return (idx_sb,
            msrc.astype(ml_dtypes.bfloat16),
            ind.astype(ml_dtypes.bfloat16))


# ---------------- device kernel ----------------
def _emit(nc):
    yp = nc.dram_tensor("yp", [SPC, T, C], F32, kind="ExternalInput")
    idx_d = nc.dram_tensor("idx", [128, 8 * (NIDX // 16)], I16,
                           kind="ExternalInput")
    msrc_d = nc.dram_tensor("msrc", [R, W], BF16, kind="ExternalInput")
    ind_d = nc.dram_tensor("ind", [R, W], BF16, kind="ExternalInput")
    out_d = nc.dram_tensor("loss_out", [SPC, 1], F32, kind="ExternalOutput")

    with tile.TileContext(nc) as tc, ExitStack() as ctx:
        consts = ctx.enter_context(tc.tile_pool(name="consts", bufs=1))
        stage_p = ctx.enter_context(tc.tile_pool(name="stage", bufs=1))
        # one gout/gbf buffer per stream iteration: zero WAR reuse, so the
        # GPSIMD/ACT stream instructions never wait on downstream consumers
        gout_p = ctx.enter_context(tc.tile_pool(name="gout", bufs=NLOAD))
        gbf_p = ctx.enter_context(tc.tile_pool(name="gbf", bufs=NLOAD))
        pstore_p = ctx.enter_context(tc.tile_pool(name="pstore", bufs=NGRP))
        state_p = ctx.enter_context(tc.tile_pool(name="state", bufs=1))
        tmp_p = ctx.enter_context(tc.tile_pool(name="tmp", bufs=3))

        idx_sb = consts.tile([128, 8 * (NIDX // 16)], I16)
        msrc = consts.tile([R, W], BF16)
        ind = consts.tile([R, W], BF16)
        nc.sync.dma_start(idx_sb[:, :], idx_d.ap())
        nc.sync.dma_start(msrc[:, :], msrc_d.ap())
        nc.sync.dma_start(ind[:, :], ind_d.ap())

        # phase-group stores: group g holds slots [32g, 32g+32) x W
        pgroup = [pstore_p.tile([R, JB * 16 * W], BF16, tag="pstore",
                                name=f"pg{g}") for g in range(NGRP)]
        # compact odd-state (label) probs and blank columns per group
        pcomp = [pstore_p.tile([R, JB * 16 * WL], BF16, tag="pcomp",
                               name=f"pc{g}") for g in range(NGRP)]
        pblank = [pstore_p.tile([R, JB * 16], F32, tag="pblank",
                                name=f"pb{g}") for g in range(NGRP)]

        # persistent stage buffers; zero pad columns memset once
        stages = [stage_p.tile([128, JB * NELEM], F32, name=f"stg{i}")
                  for i in range(NSTAGE)]
        for st in stages:
            zv = st[:, :].rearrange("p (j c) -> p j c", j=JB, c=NELEM)
            nc.gpsimd.memset(zv[:, :, ZCOL:NELEM], 0.0)

        # ---- stream: load -> gather -> cast -> collapse ----
        for g in range(NGRP):
            for o in range(8):
                it = g * 8 + o
                st = stages[it % NSTAGE]
                for j in range(JB):
                    if o < 4:
                        # fwd rows: samples 8o..8o+8, t ascending from 32g
                        t0 = JB * 16 * g + 16 * j
                        src = yp.ap()[8 * o: 8 * o + 8, t0: t0 + 16, :]
                    else:
                        # bwd rows: samples 8(o-4).., t desc. from 255-32g-16j
                        b0 = 8 * (o - 4)
                        hi = T - 1 - JB * 16 * g - 16 * j
                        src = yp.ap()[b0: b0 + 8, hi: hi - 16: -1, :]
                    nc.sync.dma_start(
                        st[:, j * NELEM: j * NELEM + C], src)
                go = gout_p.tile([128, NIDX], F32, tag="gout")
                nc.gpsimd.ap_gather(
                    out_ap=go[:, :], in_ap=st[:, :],
                    idxs_ap=idx_sb[:, (NIDX // 16) * o: (NIDX // 16) * (o + 1)],
                    channels=128, num_elems=JB * NELEM, d=1, num_idxs=NIDX)
                gb = gbf_p.tile([128, NIDX], BF16, tag="gbf")
                nc.scalar.activation(gb[:, :], go[:, :], AF.Copy, bias=EPS)
                # collapse [128, (j c)] -> rows ro..ro+8; within-group slot
                # position w' = 2*ti + j keeps (j c) contiguous on the dst
                # so the DMA stays 3-dim.
                ro = 8 * (o % 4) + (0 if o < 4 else SPC)
                dst = pcomp[g][ro: ro + 8, :].rearrange(
                    "s (ti jc) -> s ti jc", ti=16, jc=JB * WL)
                nc.scalar.dma_start(dst, gb[:, :])
                # blank (class 511) column per (row, slot): plain DMA
                stv = st[:, :].rearrange("p (j c) -> p j c", j=JB, c=NELEM)
                bdst = pblank[g][ro: ro + 8, :].rearrange(
                    "s (ti j) -> s ti j", ti=16, j=JB)
                nc.scalar.dma_start(bdst, stv[:, :, BLANK: BLANK + 1])
            # expand group: interleave labels (odd) and broadcast blank
            # (even) into the recursion slot layout, eps fused on the blank
            # path (labels got eps in the cast).
            v = pgroup[g][:, :].rearrange("r (sl k) -> r sl k",
                                          sl=JB * 16, k=W)
            pcv = pcomp[g][:, :].rearrange("r (sl k) -> r sl k",
                                           sl=JB * 16, k=WL)
            nc.scalar.activation(v[:, :, 1::2], pcv[:, :, :], AF.Copy,
                                 bias=0.0)
            pbv = pblank[g][:, :].unsqueeze(2).broadcast_to([R, JB * 16, WL])
            nc.scalar.activation(v[:, :, 0:2 * WL:2], pbv, AF.Copy, bias=EPS)

        # ---- recursion state ----
        bufA = state_p.tile([R, 2 + W], BF16)
        bufB = state_p.tile([R, 2 + W], BF16)
        racc = state_p.tile([R, 1], F32)
        mx = state_p.tile([R, 1], F32)
        rinv = state_p.tile([R, 1], F32)
        ebi = state_p.tile([R, 1], U32)
        ebf = state_p.tile([R, 1], F32)
        bconst = state_p.tile([R, 1], F32)
        nc.vector.memset(bconst[:, :], float(163 * LN2))

        nc.vector.memset(bufA[:, :], 0.0)
        nc.vector.memset(bufB[:, :], 0.0)
        nc.vector.memset(racc[:, :], 0.0)

        # init: x = slot-0 p-vector * ind
        nc.vector.tensor_tensor(bufA[:, 2:2 + W], pgroup[0][:, 0:W],
                                ind[:, :], ALU.mult)

        bufs = (bufA, bufB)
        scale_slots = {k + 2 for k in RENORM_SLOTS}
        for k in range(NSLOT):
            src_b = bufs[k % 2]
            dst_b = bufs[1 - k % 2]
            blk = k + 1
            gph, w = divmod(blk, JB * 16)
            bi = 2 * (w % 16) + (w // 16)   # w' = 2*ti + j slot order
            psl = pgroup[gph][:, bi * W: bi * W + FD]
            t_t = tmp_p.tile([R, FD], BF16, tag="t")
            u_t = tmp_p.tile([R, FD], BF16, tag="u")
            w_t = tmp_p.tile([R, FD], BF16, tag="w")
            nc.vector.tensor_tensor(t_t[:, :], src_b[:, 0:FD],
                                    msrc[:, 0:FD], ALU.mult)
            nc.vector.tensor_tensor(u_t[:, :], src_b[:, 2:2 + FD],
                                    src_b[:, 1:1 + FD], ALU.add)
            nc.vector.tensor_tensor(w_t[:, :], u_t[:, :], t_t[:, :], ALU.add)
            if k in scale_slots:
                # fold the pending renorm scale into the p-multiply
                nc.vector.scalar_tensor_tensor(dst_b[:, 2:2 + FD], w_t[:, :],
                                               rinv[:, :], psl,
                                               ALU.mult, ALU.mult)
            else:
                nc.vector.tensor_tensor(dst_b[:, 2:2 + FD], w_t[:, :], psl,
                                        ALU.mult)
            if k in RENORM_SLOTS:
                # pow2 bookkeeping: eb = raw exponent field of mx; racc
                # accumulates raw eb (the 163 = 127+36 renorm-target bias is
                # removed once at the end); rinv = exp(-ln2*(eb-163)) via the
                # Exp bias, applied at slot k+2. Renorm target 2^36 keeps the
                # in-row dynamic range clear of bf16 flush-to-zero.
                nc.vector.tensor_reduce(mx[:, :], dst_b[:, 2:2 + FD],
                                        axis=mybir.AxisListType.X, op=ALU.max)
                nc.vector.tensor_scalar(ebi[:, :],
                                        mx[:, :].bitcast(U32),
                                        23, None, ALU.logical_shift_right)
                nc.vector.tensor_copy(ebf[:, :], ebi[:, :])
                nc.vector.tensor_tensor(racc[:, :], racc[:, :], ebf[:, :],
                                        ALU.add)
                nc.scalar.activation(rinv[:, :], ebf[:, :], AF.Exp,
                                     scale=-LN2, bias=bconst[:, :])

        fin = bufs[NSLOT % 2]          # holds a_127 (fwd) / g_128 (bwd)

        # ---- combine ----
        zt = tmp_p.tile([SPC, FD], BF16, tag="t")
        zu = tmp_p.tile([SPC, FD], BF16, tag="u")
        zz = tmp_p.tile([SPC, FD], BF16, tag="w")
        nc.vector.tensor_tensor(zt[:, :], fin[0:SPC, 0:FD],
                                msrc[0:SPC, 0:FD], ALU.mult)
        nc.vector.tensor_tensor(zu[:, :], fin[0:SPC, 2:2 + FD],
                                fin[0:SPC, 1:1 + FD], ALU.add)
        nc.vector.tensor_tensor(zz[:, :], zu[:, :], zt[:, :], ALU.add)

        grev = state_p.tile([SPC, S], BF16)
        raccB = state_p.tile([SPC, 1], F32)
        # reversed copy of bwd rows into fwd partitions: grev[b, s] = g[b, 128-s]
        nc.sync.dma_start(grev[:, :], fin[SPC:R, 2 + 128: 2 - 1: -1])
        nc.sync.dma_start(raccB[:, :], racc[SPC:R, :])

        # log-space combine with exponent/mantissa decomposition (the ACT
        # Ln LUT clamps below ~1e-20, so ln args must stay in [1, 2)):
        #   ln v = LN2 * (expbits(v) - 127) + Ln(mantissa(v))
        # zero entries get a -1e18 penalty so they drop out of logsumexp.
        def exact_ln(src_bf16, pname):
            ebu = state_p.tile([SPC, S], mybir.dt.uint16, name=f"{pname}_ebu")
            ebv = state_p.tile([SPC, S], F32, name=f"{pname}_eb")
            mnt = state_p.tile([SPC, S], mybir.dt.uint16, name=f"{pname}_mn")
            lnm = state_p.tile([SPC, S], F32, name=f"{pname}_lnm")
            pen = state_p.tile([SPC, S], F32, name=f"{pname}_pen")
            lnv = state_p.tile([SPC, S], F32, name=f"{pname}_ln")
            bits = src_bf16.bitcast(mybir.dt.uint16)
            nc.vector.tensor_scalar(ebu[:, :], bits, 7, None,
                                    ALU.logical_shift_right)
            nc.vector.tensor_copy(ebv[:, :], ebu[:, :])
            nc.vector.tensor_scalar(mnt[:, :], bits, 0x7F, None,
                                    ALU.bitwise_and)
            nc.vector.tensor_scalar(mnt[:, :], mnt[:, :], 0x3F80, None,
                                    ALU.bitwise_or)
            nc.scalar.activation(lnm[:, :], mnt[:, :].bitcast(BF16), AF.Ln)
            nc.vector.tensor_scalar(pen[:, :], bits, 0, -1e18,
                                    ALU.is_equal, ALU.mult)
            nc.vector.tensor_scalar(ebv[:, :], ebv[:, :], LN2, -127.0 * LN2,
                                    ALU.mult, ALU.add)
            nc.vector.tensor_tensor(lnv[:, :], ebv[:, :], lnm[:, :], ALU.add)
            nc.vector.tensor_tensor(lnv[:, :], lnv[:, :], pen[:, :], ALU.add)
            return lnv

        lnz = exact_ln(zz[:, 0:S], "z")
        lng = exact_ln(grev[:, :], "g")
        sums = state_p.tile([SPC, S], F32)
        m_t = state_p.tile([SPC, 1], F32)
        negm = state_p.tile([SPC, 1], F32)
        e_t = state_p.tile([SPC, S], F32)
        dot = state_p.tile([SPC, 1], F32)
        nc.vector.tensor_tensor(sums[:, :], lnz[:, :], lng[:, :], ALU.add)
        nc.vector.tensor_scalar_max(sums[:, :], sums[:, :], -1e18)
        nc.vector.tensor_reduce(m_t[:, :], sums[:, :],
                                axis=mybir.AxisListType.X, op=ALU.max)
        nc.vector.tensor_scalar_mul(negm[:, :], m_t[:, :], -1.0)
        nc.scalar.activation(e_t[:, :], sums[:, :], AF.Exp, bias=negm[:, :])
        nc.vector.tensor_reduce(dot[:, :], e_t[:, :],
                                axis=mybir.AxisListType.X, op=ALU.add)
        lnd = state_p.tile([SPC, 1], F32)
        nc.scalar.activation(lnd[:, :], dot[:, :], AF.Ln)
        # loss = -(ln(dot) + m + LN2*(raccF + raccB))
        s1 = state_p.tile([SPC, 1], F32)
        s2 = state_p.tile([SPC, 1], F32)
        s3 = state_p.tile([SPC, 1], F32)
        loss = state_p.tile([SPC, 1], F32)
        nc.vector.tensor_tensor(s1[:, :], racc[0:SPC, :], raccB[:, :], ALU.add)
        # remove the 163 renorm-target bias for all 2*NREN accumulations
        nc.vector.tensor_scalar(s1[:, :], s1[:, :], LN2,
                                -2.0 * len(RENORM_SLOTS) * 163 * LN2,
                                ALU.mult, ALU.add)
        nc.vector.tensor_tensor(s2[:, :], lnd[:, :], m_t[:, :], ALU.add)
        nc.vector.tensor_tensor(s3[:, :], s2[:, :], s1[:, :], ALU.add)
        nc.vector.tensor_scalar_mul(loss[:, :], s3[:, :], -1.0)
        nc.sync.dma_start(out_d.ap(), loss[:, :])
    return nc


_NC_CACHE = None


def _build():
    global _NC_CACHE
    if _NC_CACHE is None:
        nc = bacc.Bacc("TRN2", target_bir_lowering=False, debug=False,
                       enable_asserts=False)
        _emit(nc)
        nc.compile()
        _NC_CACHE = nc
    return _NC_CACHE


def kernel(y_true, y_pred):
    y_true = np.asarray(y_true)
    y_pred = np.ascontiguousarray(np.asarray(y_pred, np.float32))
    nc = _build()
    in_maps = []
    for c in range(NCORES):
        sl = slice(c * SPC, (c + 1) * SPC)
        idx_sb, msrc, ind = _core_tables(y_true[sl])
        in_maps.append(dict(yp=y_pred[sl], idx=idx_sb, msrc=msrc, ind=ind))
    res = run_bass_kernel_spmd(nc, in_maps, core_ids=list(range(NCORES)))
    loss = np.concatenate([res.results[c]["loss_out"] for c in range(NCORES)],
                          axis=0)
    return loss.astype(np.float32)
